# revision 42
# baseline (speedup 1.0000x reference)
"""Trainium2 Bass kernel for nn_Attention_LoRA_FFT.

Sharding: data-parallel over batch B=8 across the 8 NeuronCores. The DCT
LoRA weight reconstruction is sharded: each core builds a 256-column slice
of one of WkT/WvT (chosen by per-core input data, the program is identical)
and a bf16 AllGather distributes the full weights.

Everything on-device runs in bf16 (psum accumulation stays f32): same PE
cycles/row as f32r but half the DMA / SBUF / collective bytes.

Per-core device program:
  warm) dummy matmuls during the first DMA wait keep the PE HAM clock-gate
        warm; a tiny exp preloads the ACT table set.
  A) G = Sw.T @ Bmq ; Wpart = Bm.T @ G       (bf16, 1/8 of the work)
     AllGather (x2 column halves) -> full WkT, WvT
  B) qT = W_q @ x.T                          (covers the collective)
     fold: W_k' = W_k + WkT, W_v' = W_v + WvT (DVE adds), then
     kT = W_k' @ x.T  (even 128-col blocks first: they only need gather 0)
     V' = [x @ W_v'.T | 1]
     All psum->sbuf copies on DVE; ACT only ever runs exps.
  C) per head pair: S.T = kT_h.T @ qT_h      (row-packed pairs: concurrent)
     P.T = exp(S.T/8)  (ACT, psum->bf16; no max-subtraction: scores O(10))
     [O.T ; Z] = V'.T @ P.T  (ones col gives Z) ; O.T *= bcast(1/Z)
     stage1 of the first units is interleaved into the V' build; the steady
     loop interleaves stage1(u+2)/stage2(u)/proj at 4-matmul granularity.
  D) y.T = W_proj @ O.T + b                  -> DMA out, host transposes
"""

import os
import sys

for _p in ("/opt/trn_rl_repo", "/root/.axon_site/_ro/trn_rl_repo"):
    if os.path.isdir(_p) and _p not in sys.path:
        sys.path.insert(0, _p)

import numpy as np

import concourse.bacc as bacc
import concourse.mybir as mybir
from concourse.tile import TileContext
from concourse.bass_utils import run_bass_kernel_spmd

B, N, C = 8, 1024, 1024
H, HD = 16, 64
NCORES = 8
PC = C // 128
F32 = mybir.dt.float32
BF16 = mybir.dt.bfloat16
EXP = mybir.ActivationFunctionType.Exp


def _dct_matrix(n: int) -> np.ndarray:
    i = np.arange(n, dtype=np.float32)[:, None]
    j = np.arange(n, dtype=np.float32)[None, :]
    m = np.sqrt(np.float32(2.0 / n)) * np.cos(
        np.float32(np.pi) * i * (2.0 * j + 1.0) / np.float32(2.0 * n)
    )
    m[0, :] = np.sqrt(np.float32(1.0 / n))
    return m.astype(np.float32)


def _build():
    nc = bacc.Bacc("TRN2", target_bir_lowering=False, debug=False, num_devices=NCORES)

    xT_d = nc.dram_tensor("xT", [C, N], BF16, kind="ExternalInput")
    wqkvT_d = nc.dram_tensor("wqkvT", [C, 3 * C], BF16, kind="ExternalInput")
    wprojT_d = nc.dram_tensor("wprojT", [C, C], BF16, kind="ExternalInput")
    bias_d = nc.dram_tensor("bias", [C, 1], F32, kind="ExternalInput")
    bm_d = nc.dram_tensor("bm", [C, C], BF16, kind="ExternalInput")
    sw_d = nc.dram_tensor("sw", [C, C], BF16, kind="ExternalInput")
    bmq_d = nc.dram_tensor("bmq", [C, 256], BF16, kind="ExternalInput")
    yT_d = nc.dram_tensor("yT", [C, N], F32, kind="ExternalOutput")
    cc_in0 = nc.dram_tensor("cc_in0", [C, 128], BF16)
    cc_in1 = nc.dram_tensor("cc_in1", [C, 128], BF16)
    cc_out0 = nc.dram_tensor("cc_out0", [NCORES * C, 128], BF16, addr_space="Shared")
    cc_out1 = nc.dram_tensor("cc_out1", [NCORES * C, 128], BF16, addr_space="Shared")

    def col_slab(dram_ap, pool, tag, f0, width, queue):
        slab = pool.tile([128, PC, width], BF16, tag=tag, name=tag)
        queue.dma_start(
            out=slab[:],
            in_=dram_ap[:, f0 : f0 + width].rearrange("(cc p) f -> p cc f", p=128),
        )
        return slab

    with TileContext(nc) as tc:
        # ---------------- persistent left stack ----------------
        # release order (wk, then x, then wv) is the reverse of alloc order
        # so the left stack pops cleanly mid-kernel.
        small_p = tc.alloc_tile_pool(name="small", bufs=1, side="left")
        qt_p = tc.alloc_tile_pool(name="qtp", bufs=1, side="left")
        kt_p = tc.alloc_tile_pool(name="ktp", bufs=1, side="left")
        vp_p = tc.alloc_tile_pool(name="vpp", bufs=1, side="left")
        ot_p = tc.alloc_tile_pool(name="otp", bufs=1, side="left")
        wv_p = tc.alloc_tile_pool(name="wvp", bufs=1, side="left")
        x_p = tc.alloc_tile_pool(name="xp", bufs=1, side="left")
        wk_p = tc.alloc_tile_pool(name="wkp", bufs=1, side="left")

        bias_sb = small_p.tile([128, PC, 1], F32, tag="bias")
        nc.sync.dma_start(
            out=bias_sb[:], in_=bias_d.rearrange("(cc p) o -> p cc o", p=128)
        )

        x_sb = x_p.tile([128, PC, N], BF16, tag="x")
        qT_sb = qt_p.tile([128, PC, N], BF16, tag="qT")
        kT_sb = kt_p.tile([128, PC, N], BF16, tag="kT")
        vp_sb = vp_p.tile([128, PC, H, HD + 1], BF16, tag="vp")
        oT_sb = ot_p.tile([128, PC, N], BF16, tag="oT")
        wk_sb = wk_p.tile([128, PC, C], BF16, tag="wk")
        wv_sb = wv_p.tile([128, PC, C], BF16, tag="wv")

        # ---- PE warm-up during the initial DMA wait; ACT exp-table load ----
        warm_sb = small_p.tile([128, 512], BF16, tag="warm")
        nc.vector.memset(warm_sb[:], 0.125)
        scr_sb = small_p.tile([128, 8], F32, tag="scr")
        nc.scalar.activation(scr_sb[:], warm_sb[:, 0:8], EXP)
        psW = tc.alloc_tile_pool(name="psw", bufs=1, space="PSUM")
        ps_warm = psW.tile([128, 512], F32, tag="w", name="ps_warm")
        for i in range(10):
            nc.tensor.matmul(
                ps_warm[:],
                warm_sb[:, 0:128],
                warm_sb[:],
                start=(i == 0),
                stop=(i == 9),
            )

        # ================= Phase A: sharded LoRA reconstruction =======
        slabA_p = tc.alloc_tile_pool(name="slabA", bufs=3, side="right")
        bm_p = tc.alloc_tile_pool(name="bmp", bufs=1, side="right")
        bmq_p = tc.alloc_tile_pool(name="bmqp", bufs=1, side="right")
        g_p = tc.alloc_tile_pool(name="gp", bufs=1, side="right")
        wpart_p = tc.alloc_tile_pool(name="wpartp", bufs=1, side="right")
        psA = tc.alloc_tile_pool(name="psA", bufs=2, space="PSUM")

        bmq_sb = bmq_p.tile([128, PC, 256], BF16, tag="bmq")
        nc.scalar.dma_start(
            out=bmq_sb[:], in_=bmq_d.rearrange("(cc p) f -> p cc f", p=128)
        )
        bm_sb = bm_p.tile([128, PC, C], BF16, tag="bm")

        g_sb = g_p.tile([128, PC, 256], BF16, tag="g", name="g_sb")
        wpart_sb = wpart_p.tile([128, PC, 256], BF16, tag="wpart", name="wpart_sb")
        for at in range(PC):
            slab = col_slab(sw_d, slabA_p, "slabA", at * 128, 128, nc.scalar)
            ps = psA.tile([128, 256], F32, tag="psA", name="psA_t")
            for bc in range(PC):
                nc.tensor.matmul(
                    ps[:],
                    slab[:, bc, :],
                    bmq_sb[:, bc, :],
                    start=(bc == 0),
                    stop=(bc == PC - 1),
                )
            nc.scalar.copy(g_sb[:, at, :], ps[:])
            # stream bm in behind the sw slabs on the same queue
            nc.scalar.dma_start(
                out=bm_sb[:, at, :],
                in_=bm_d[at * 128 : (at + 1) * 128, :],
            )
        for ct in range(PC):
            ps = psA.tile([128, 256], F32, tag="psA", name="psA_t")
            for ac in range(PC):
                nc.tensor.matmul(
                    ps[:],
                    bm_sb[:, ac, ct * 128 : (ct + 1) * 128],
                    g_sb[:, ac, :],
                    start=(ac == 0),
                    stop=(ac == PC - 1),
                )
            nc.scalar.copy(wpart_sb[:, ct, :], ps[:])

        for hf, cc_in in ((0, cc_in0), (1, cc_in1)):
            nc.scalar.dma_start(
                out=cc_in.rearrange("(ct p) f -> p ct f", p=128),
                in_=wpart_sb[:, :, hf * 128 : (hf + 1) * 128],
            )

        # AllGather the WkT/WvT quarters in two column-halves; read back on
        # the gpsimd DMA queue so the sync queue keeps streaming weight slabs.
        for hf, cc_in, cc_out in ((0, cc_in0, cc_out0), (1, cc_in1, cc_out1)):
            nc.gpsimd.collective_compute(
                "AllGather",
                mybir.AluOpType.bypass,
                replica_groups=[list(range(NCORES))],
                ins=[cc_in[:]],
                outs=[cc_out[:]],
            )
            for wi, w_sb in ((0, wk_sb), (1, wv_sb)):
                for fq in range(4):
                    base = (wi * 4 + fq) * C
                    nc.gpsimd.dma_start(
                        out=w_sb[
                            :, :, fq * 256 + hf * 128 : fq * 256 + (hf + 1) * 128
                        ],
                        in_=cc_out[base : base + C, :].rearrange(
                            "(cc p) f -> p cc f", p=128
                        ),
                    )

        # x arrives on the sync queue while phase A computes
        for cc in range(PC):
            nc.sync.dma_start(
                out=x_sb[:, cc, :], in_=xT_d[cc * 128 : (cc + 1) * 128, :]
            )

        psA.release()
        psW.release()
        wpart_p.release()
        g_p.release()
        bmq_p.release()
        bm_p.release()
        slabA_p.release()

        # ================= Phase C machinery (defined early: stage1 of the
        # first units interleaves into the phase-B loops) ================
        pt_p = tc.alloc_tile_pool(name="ptp", bufs=3, side="right")
        rz_p = tc.alloc_tile_pool(name="rzp", bufs=2, side="right")
        or_p = tc.alloc_tile_pool(name="orp", bufs=2, side="right")
        zb_p = tc.alloc_tile_pool(name="zbp", bufs=2, side="right")
        psS = tc.alloc_tile_pool(name="psS", bufs=1, space="PSUM")
        psO = tc.alloc_tile_pool(name="psO", bufs=1, space="PSUM")

        scale = float(HD) ** -0.5
        units = [(ih, hp) for ih in range(2) for hp in range(H // 2)]
        staged = {}
        ps_big = psS.tile([128, 4, 512], F32, tag="sbig", name="ps_big")
        slot_ctr = [0]

        def stage1_chunk(u, j0):
            """4 S matmuls (two concurrent 64-row pairs) + 2 exps for
            (unit u, kv chunks j0, j0+1)."""
            ih, hp = units[u]
            i0 = ih * 512
            if j0 == 0:
                staged[u] = [
                    pt_p.tile([128, PC, 512], BF16, tag=f"pt{sub}", name="pt_t")
                    for sub in range(2)
                ]
            pts = staged[u]
            slots = []
            for sub in range(2):
                s = slot_ctr[0] % 2
                slot_ctr[0] += 1
                slots.append(ps_big[:, 2 * s : 2 * s + 2, :])
            for dj in range(2):
                for sub in range(2):  # adjacent row-group pair: concurrent
                    p0 = sub * 64
                    nc.tensor.matmul(
                        slots[sub][:, dj, :],
                        kT_sb[
                            p0 : p0 + 64,
                            hp,
                            (j0 + dj) * 128 : (j0 + dj + 1) * 128,
                        ],
                        qT_sb[p0 : p0 + 64, hp, i0 : i0 + 512],
                    )
            for sub in range(2):
                nc.scalar.activation(
                    pts[sub][:, j0 : j0 + 2, :].rearrange("p j i -> p (j i)"),
                    slots[sub].rearrange("p j i -> p (j i)"),
                    EXP,
                    scale=scale,
                )

        ps_os = {}

        def stage2_quarter(u, q):
            """Quarter q of unit u's output accumulation: sub q//2, kv
            chunks 4*(q%2)..+4; the z-normalize chain follows the last
            quarter of each sub."""
            ih, hp = units[u]
            i0 = ih * 512
            sub, half = q // 2, q % 2
            h = 2 * hp + sub
            p0 = sub * 64
            pt = staged[u][sub]
            if half == 0:
                ps_os[(u, sub)] = psO.tile(
                    [HD + 1, 512], F32, tag=f"o{sub}", name="psO_t"
                )
            ps_o = ps_os[(u, sub)]
            for j in range(4 * half, 4 * half + 4):
                nc.tensor.matmul(
                    ps_o[:],
                    vp_sb[:, j, h, :],
                    pt[:, j, :],
                    start=(j == 0),
                    stop=(j == PC - 1),
                )
            if half == 1:
                # drain psO with two quick DVE copies; normalize later from
                # SBUF (DVE psum reads during exps slow the ACT down, and
                # the long in-psum chain stalls the next unit's O matmuls)
                zraw = rz_p.tile([1, 512], F32, tag="rz", name="rz_t", bufs=1)
                nc.vector.tensor_copy(zraw[:], ps_o[HD : HD + 1, :])
                oraw = or_p.tile([HD, 512], F32, tag=f"or{sub}", name="oraw_t")
                nc.vector.tensor_copy(oraw[:], ps_o[0:HD, :])
                ps_os.pop((u, sub))
                zbc = zb_p.tile([HD, 512], F32, tag="zbc", name="zbc_t", bufs=1)
                nc.gpsimd.partition_broadcast(zbc[:], zraw[:], channels=HD)
                zb = zb_p.tile([HD, 512], F32, tag="zb", name="zb_t")
                nc.vector.reciprocal_approx_fast(zb[:], zbc[:])
                nc.vector.tensor_mul(
                    oT_sb[p0 : p0 + 64, hp, i0 : i0 + 512], oraw[:], zb[:]
                )
                if sub == 1:
                    staged.pop(u)

        # ================= Phase B =====================================
        slabB_p = tc.alloc_tile_pool(name="slabB", bufs=3, side="right")
        fold_p = tc.alloc_tile_pool(name="foldp", bufs=2, side="right")
        psB = tc.alloc_tile_pool(name="psB", bufs=2, space="PSUM", side="right")

        # ---- qT (no lora dependency: covers the collective) ----
        for fc in range(PC):
            slab = col_slab(wqkvT_d, slabB_p, "slabB", fc * 128, 128, nc.sync)
            for th in range(2):
                ps = psB.tile([128, 512], F32, tag="psB", name="psB_t")
                for cc in range(PC):
                    nc.tensor.matmul(
                        ps[:],
                        slab[:, cc, :],
                        x_sb[:, cc, th * 512 : (th + 1) * 512],
                        start=(cc == 0),
                        stop=(cc == PC - 1),
                    )
                nc.vector.tensor_copy(qT_sb[:, fc, th * 512 : (th + 1) * 512], ps[:])

        # ---- dummy chains bridge the AllGather wait, keeping the PE
        # clock-gate warm (results never read; they use the still-idle
        # S-staging banks) ----
        for c in range(12):
            for i in range(8):
                nc.tensor.matmul(
                    ps_big[:, c % 4, :],
                    warm_sb[:, 0:128],
                    warm_sb[:],
                    start=(i == 0),
                    stop=(i == 7),
                )

        # ---- kT = (W_k + Wk_lora) @ x.T (even fc first: needs gather 0) ----
        for fc in [0, 2, 4, 6, 1, 3, 5, 7]:
            slab = col_slab(wqkvT_d, slabB_p, "slabB", C + fc * 128, 128, nc.sync)
            kf = fold_p.tile([128, PC, 128], BF16, tag="kf", name="kf")
            nc.vector.tensor_add(
                kf[:], slab[:], wk_sb[:, :, fc * 128 : (fc + 1) * 128]
            )
            for th in range(2):
                ps = psB.tile([128, 512], F32, tag="psB", name="psB_t")
                for cc in range(PC):
                    nc.tensor.matmul(
                        ps[:],
                        kf[:, cc, :],
                        x_sb[:, cc, th * 512 : (th + 1) * 512],
                        start=(cc == 0),
                        stop=(cc == PC - 1),
                    )
                nc.vector.tensor_copy(kT_sb[:, fc, th * 512 : (th + 1) * 512], ps[:])
        fold_p.release()
        slabB_p.release()

        # ---- V' = [x @ (W_v + Wv_lora).T | 1], with stage1 of the first
        # two units interleaved between the chains ----
        vf_p = tc.alloc_tile_pool(name="vfp", bufs=1, side="right")
        s1_sched = {
            2: ("s1", 0, 0), 4: ("s1", 0, 2), 6: ("s1", 0, 4), 8: ("s1", 0, 6),
            9: ("s1", 1, 0), 10: ("s1", 1, 2), 11: ("s1", 1, 4), 12: ("s1", 1, 6),
            13: ("mix", (2, 0), 0), 14: ("mix", (2, 2), 1), 15: ("mix", (2, 4), 2),
        }
        chain = 0
        for fh in range(2):
            vslab = col_slab(wqkvT_d, vf_p, "vslab", 2 * C + fh * 512, 512, nc.sync)
            vfold = vf_p.tile([128, PC, 512], BF16, tag="vfold", name="vfold")
            nc.vector.tensor_add(
                vfold[:], vslab[:], wv_sb[:, :, fh * 512 : (fh + 1) * 512]
            )
            for tc_i in range(PC):
                ps = psB.tile([128, 512], F32, tag="psB", name="psB_t")
                for cc in range(PC):
                    nc.tensor.matmul(
                        ps[:],
                        x_sb[:, cc, tc_i * 128 : (tc_i + 1) * 128],
                        vfold[:, cc, :],
                        start=(cc == 0),
                        stop=(cc == PC - 1),
                    )
                nc.vector.tensor_copy(
                    vp_sb[:, tc_i, fh * 8 : (fh + 1) * 8, 0:HD],
                    ps[:].rearrange("p (h d) -> p h d", d=HD),
                )
                nc.vector.memset(
                    vp_sb[:, tc_i, fh * 8 : (fh + 1) * 8, HD : HD + 1], 1.0
                )
                if chain in s1_sched:
                    ev = s1_sched[chain]
                    if ev[0] == "s1":
                        stage1_chunk(ev[1], ev[2])
                    else:
                        stage2_quarter(0, ev[2])
                        stage1_chunk(*ev[1])
                chain += 1
        vf_p.release()
        wk_p.release()
        x_p.release()
        wv_p.release()

        # ---- output projection helper ----
        wps_p = tc.alloc_tile_pool(name="wpsp", bufs=4, side="right")
        y_p = tc.alloc_tile_pool(name="yp", bufs=3, side="right")
        proj_state = [None]

        def proj_part(fo, th, half):
            """Half of one output-projection group (4 of 8 matmuls)."""
            if half == 0:
                wps = wps_p.tile([128, PC, 128], BF16, tag="wps", name="wps")
                nc.sync.dma_start(
                    out=wps[:],
                    in_=wprojT_d[:, fo * 128 : (fo + 1) * 128].rearrange(
                        "(cc p) f -> p cc f", p=128
                    ),
                )
                ps = psB.tile([128, 512], F32, tag="psB", name="psB_t")
                proj_state[0] = (wps, ps)
            wps, ps = proj_state[0]
            for cc in range(4 * half, 4 * half + 4):
                nc.tensor.matmul(
                    ps[:],
                    wps[:, cc, :],
                    oT_sb[:, cc, th * 512 : (th + 1) * 512],
                    start=(cc == 0),
                    stop=(cc == PC - 1),
                )
            if half == 1:
                y_sb = y_p.tile([128, 512], F32, tag="y", name="y_t")
                nc.vector.tensor_scalar_add(y_sb[:], ps[:], bias_sb[:, fo, :])
                nc.scalar.dma_start(
                    out=yT_d[fo * 128 : (fo + 1) * 128, th * 512 : (th + 1) * 512],
                    in_=y_sb[:],
                )

        # ---- steady state: stage1(u+2) / stage2(u) / proj interleaved ----
        stage1_chunk(2, 6)
        stage2_quarter(0, 3)
        nunits = len(units)
        for u2 in range(1, nunits):
            for q in range(4):
                stage2_quarter(u2, q)
                # proj for the first i-half interleaves once its oT is done
                if u2 >= 8 and q in (1, 3):
                    proj_part(u2 - 8, 0, (q - 1) // 2)
                if u2 + 2 < nunits:
                    stage1_chunk(u2 + 2, 2 * q)

        # ---- remaining projection half ----
        for fo in range(PC):
            proj_part(fo, 1, 0)
            proj_part(fo, 1, 1)

        y_p.release()
        wps_p.release()
        zb_p.release()
        or_p.release()
        rz_p.release()
        pt_p.release()
        psO.release()
        psS.release()
        psB.release()
        ot_p.release()
        vp_p.release()
        kt_p.release()
        qt_p.release()
        small_p.release()

    nc.compile()
    return nc


_CACHE = {}


def _get_nc():
    if "nc" not in _CACHE:
        _CACHE["nc"] = _build()
    return _CACHE["nc"]


def _host_prep(x, W_qkv, W_proj, b_proj, coef_k, coef_v, indices, task):
    import ml_dtypes

    bf16 = ml_dtypes.bfloat16
    x = np.asarray(x, dtype=np.float32)
    W_qkv = np.asarray(W_qkv, dtype=np.float32)
    W_proj = np.asarray(W_proj, dtype=np.float32)
    b_proj = np.asarray(b_proj, dtype=np.float32)
    coef_k = np.asarray(coef_k, dtype=np.float32)
    coef_v = np.asarray(coef_v, dtype=np.float32)
    indices = np.asarray(indices)
    t = int(np.asarray(task).reshape(())) + 1

    assert x.shape == (B, N, C), x.shape

    # Host-side input marshaling: scatter the per-task frequency coefficients
    # into dense C x C planes (the sum across tasks commutes with the linear
    # inverse DCT), exactly as the reference does before its matmuls.
    def scatter(coef, idx):
        s = np.zeros(C * C, dtype=np.float32)
        np.add.at(s, idx.reshape(-1).astype(np.int64), coef.reshape(-1))
        return s.reshape(C, C)

    bm = _dct_matrix(C)
    sk = scatter(coef_k[:t], indices[:t]).astype(bf16)
    sv = scatter(coef_v[:t], indices[:t]).astype(bf16)
    bm16 = bm.astype(bf16)

    shared = {
        "wqkvT": np.ascontiguousarray(W_qkv.T).astype(bf16),
        "wprojT": np.ascontiguousarray(W_proj.T).astype(bf16),
        "bias": np.ascontiguousarray(b_proj.reshape(C, 1)),
        "bm": bm16,
    }
    maps = []
    for b in range(NCORES):
        fq = b % 4
        maps.append(
            {
                "xT": np.ascontiguousarray(x[b].T).astype(bf16),
                "sw": sk if b < 4 else sv,
                "bmq": np.ascontiguousarray(bm16[:, fq * 256 : (fq + 1) * 256]),
                **shared,
            }
        )
    return maps


def kernel(x, W_qkv, W_proj, b_proj, coef_k, coef_v, indices, task):
    in_maps = _host_prep(x, W_qkv, W_proj, b_proj, coef_k, coef_v, indices, task)
    nc = _get_nc()
    res = run_bass_kernel_spmd(nc, in_maps, list(range(NCORES)))

    out = np.empty((B, N, C), dtype=np.float32)
    for b in range(NCORES):
        out[b] = res.results[b]["yT"].T
    return out
